# revision 26
# baseline (speedup 1.0000x reference)
"""Multi-head attention (b=2, n=2048, dim=1024, h=16, fp32) on 8 TRN2 NeuronCores.

Sharding: 2 batches x 4 head-groups (4 heads each). Each core computes, for its
batch element and 4 heads: QKV projection, softmax attention, and a partial
output projection (W_out rows of its heads). Host sums the 4 partials per batch
and adds the bias.

Device layout choices (per core):
  - x arrives pre-transposed (host) as xT [128, 8, 2048]  (p=dim%128, kc=dim//128, n)
  - Q^T/K^T computed as [128, 2, 2048]: pair g holds heads 2g (partitions 0-63)
    and 2g+1 (partitions 64-127); row r of pair g = W_qkv column (g*128+r).
  - S^T = K @ Q^T per head via row-tiled (K=64) matmul pairs; softmax exp on
    ScalarE directly PSUM->SBUF with scale=dim^-0.5 folded in (no max
    subtraction needed: |scores*scale| < ~0.5).
  - V is augmented with a ones column per head ([V_h | 1]) so the PV matmul's
    65th output row accumulates the softmax denominator for free.
  - Normalization: reciprocal_approx_fast (DVE) + partition_broadcast (GPSIMD)
    + one tensor_tensor multiply fused with the PSUM->SBUF evacuation.
  - Matmuls run as float32r (TF32-like single-pass, 4x faster than fp32).
"""

import os
import numpy as np
from contextlib import ExitStack

import concourse.bass as bass
import concourse.mybir as mybir
import concourse.tile as tile
from concourse import bacc
from concourse.bass import ts
from concourse.bass_utils import run_bass_kernel_spmd

F32 = mybir.dt.float32
F32R = mybir.dt.float32r

N_CORES = 8
HEADS = 16
DH = 64  # head dim


class Cfg:
    def __init__(self, n, dim, hg):
        self.n = n                    # sequence length (per core)
        self.dim = dim                # model dim
        self.hg = hg                  # heads per core
        self.kc = dim // 128          # dim chunks of 128
        self.nqb = max(1, n // 512)   # query blocks of 512
        self.qb = min(n, 512)
        self.nkc = n // 128           # key chunks of 128
        self.pairs = hg // 2
        self.shard = hg * DH          # qkv shard columns per section
        self.vw = hg * (DH + 1)       # V columns incl per-head ones col
        self.mm_dt = F32R
        self.np_dt = np.float32
        # S-psum group sizes (banks per exp activate); sum == nkc
        g, rem, sizes = 3, n // 128, []
        while rem > 0:
            sizes.append(min(g, rem))
            rem -= min(g, rem)
        self.groups = sizes


FULL = Cfg(2048, 1024, 4)
# fp16 matmuls: 1 cycle/column streaming + FWL weight loads (fp32r streams at
# ~2 cyc/col). Measured end-to-end relative error 4.1e-4 vs the fp32
# reference (fp32r variant: 1.8e-4 at ~15% more time). Set ATTN_MM_DT=fp32r
# to switch back.
if os.environ.get("ATTN_MM_DT", "fp16") == "fp16":
    FULL.mm_dt = mybir.dt.float16
    FULL.np_dt = np.float16


def build_kernel(tc, ctx, cfg, xT, wq, wk, wv, wo, out, dbg=None):
    nc = tc.nc
    P = 128
    KC, NQB, QB, NKC, PAIRS = cfg.kc, cfg.nqb, cfg.qb, cfg.nkc, cfg.pairs
    MD = cfg.mm_dt
    SCALE = cfg.dim ** -0.5
    M_SLABS = cfg.shard // 128  # = PAIRS

    def r(ap):
        return ap

    wpool = ctx.enter_context(tc.tile_pool(name="w", bufs=1))
    wq_sb = wpool.tile([P, KC, cfg.shard], MD, tag="wq", name="wq_sb")
    wk_sb = wpool.tile([P, KC, cfg.shard], MD, tag="wk", name="wk_sb")
    wv_sb = wpool.tile([P, KC, cfg.vw], MD, tag="wv", name="wv_sb")
    wo_sb = wpool.tile([P, M_SLABS, cfg.dim], MD, tag="wo", name="wo_sb")

    per = ctx.enter_context(tc.tile_pool(name="per", bufs=1))
    qt = {}  # (pair, nqb) -> [128, QB]
    kt = {}
    vt = {}  # nt -> [128, vw]
    on = {}  # (slab, nqb) -> [128, QB]  normalized O^T for out-proj lhsT
    for g in range(PAIRS):
        for b in range(NQB):
            qt[g, b] = per.tile([P, QB], MD, tag=f"qt{g}_{b}", name=f"qt{g}_{b}")
            kt[g, b] = per.tile([P, QB], MD, tag=f"kt{g}_{b}", name=f"kt{g}_{b}")
            on[g, b] = per.tile([P, QB], MD, tag=f"on{g}_{b}", name=f"on{g}_{b}")
    for t in range(NKC):
        vt[t] = per.tile([P, cfg.vw], MD, tag=f"v{t}", name=f"v{t}")

    xpool = ctx.enter_context(tc.tile_pool(name="x", bufs=1))
    paQ = ctx.enter_context(tc.tile_pool(name="paQ", bufs=2, space="PSUM"))
    psS = ctx.enter_context(tc.tile_pool(name="psS", bufs=2, space="PSUM"))
    psO = ctx.enter_context(tc.tile_pool(name="psO", bufs=1, space="PSUM"))
    epool = ctx.enter_context(tc.tile_pool(name="e", bufs=6))
    npool = ctx.enter_context(tc.tile_pool(name="nrm", bufs=3))
    copool = ctx.enter_context(tc.tile_pool(name="co", bufs=6))

    xts = {}
    for b in range(NQB):
        xts[b] = xpool.tile([P, KC, QB], MD, tag=f"xt{b}", name=f"xt{b}")
    # per-kc slices so the first K-projection chain starts after ~1 slice
    # of data instead of the full 2 MB
    for kc in range(KC):
        nc.sync.dma_start(wk_sb[:, kc : kc + 1], wk[:, kc : kc + 1])
        nc.sync.dma_start(xts[0][:, kc : kc + 1], xT[:, kc : kc + 1, ts(0, QB)])
    nc.sync.dma_start(wq_sb[:], wq[:])
    for b in range(1, NQB):
        nc.sync.dma_start(xts[b][:], xT[:, :, ts(b, QB)])
    nc.sync.dma_start(wv_sb[:], wv[:])

    def emit_qk(w_sb, dst, g, b):
        ps = paQ.tile([P, 512], F32, tag="pa", name="pa")
        for kc in range(KC):
            nc.tensor.matmul(
                ps[:, :QB],
                lhsT=w_sb[:, kc, ts(g, 128)],
                rhs=xts[b][:, kc, :],
                start=(kc == 0),
                stop=(kc == KC - 1),
            )
        nc.vector.tensor_copy(dst[g, b][:], ps[:, :QB])

    def emit_v(nt):
        vb, t = divmod(nt, QB // 128)
        ps = paQ.tile([P, 512], F32, tag="pa", name="pa")
        for kc in range(KC):
            nc.tensor.matmul(
                ps[:, : cfg.vw],
                lhsT=xts[vb][:, kc, ts(t, 128)],
                rhs=wv_sb[:, kc, :],
                start=(kc == 0),
                stop=(kc == KC - 1),
            )
        nc.vector.tensor_copy(vt[nt][:], ps[:, : cfg.vw])
        v4 = vt[nt][:].rearrange("p (h e) -> p h e", e=DH + 1)
        ones_ap = v4[:, :, DH : DH + 1]
        if mybir.dt.size(MD) == 4:
            ones_ap = ones_ap.bitcast(F32)
        nc.vector.memset(ones_ap, 1.0)

    emitted = set()

    def attention(b, g, with_v=False, fillers=None, nfill=0):
        o_ps = psO.tile([P, 2, 512], F32, tag="o", name="o_ps")
        done = 0
        for c in range(NKC):
            if with_v:
                emit_v(c)
            if fillers and done < nfill and c in (5, 9, 13):
                key, fn = fillers.popleft()
                fn()
                emitted.add(key)
                done += 1
            s_ps = psS.tile([P, 2, 512], F32, tag="s", name="s_ps")
            for a in range(2):
                lo = a * 64
                nc.tensor.matmul(
                    s_ps[:, a, :QB],
                    lhsT=kt[g, c * 128 // QB][lo : lo + 64, ts(c % (QB // 128), 128)],
                    rhs=qt[g, b][lo : lo + 64, :],
                    start=True,
                    stop=True,
                )
            e_t = epool.tile([P, 2, 512], MD, tag="e", name="e_t")
            nc.scalar.activation(
                e_t[:, :, :QB],
                s_ps[:, :, :QB],
                mybir.ActivationFunctionType.Exp,
                scale=SCALE,
            )
            if dbg is not None and b == 0 and g == 0 and c == 0:
                nc.sync.dma_start(dbg["e00"][:], e_t[:, :, :QB])
            v4 = vt[c][:].rearrange("p (h e) -> p h e", e=DH + 1)
            for a in range(2):
                h = 2 * g + a
                nc.tensor.matmul(
                    o_ps[0 : DH + 1, a, :QB],
                    lhsT=v4[:, h, :],
                    rhs=e_t[:, a, :QB],
                    start=(c == 0),
                    stop=(c == NKC - 1),
                )
        # one-shot evacuation frees the O psum banks immediately
        oev = npool.tile([P, 2, 512], F32, tag="oev", name="oev")
        nc.vector.tensor_copy(oev[0 : DH + 1, :, :QB], o_ps[0 : DH + 1, :, :QB])
        if dbg is not None and b == 0 and g == 0:
            nc.sync.dma_start(dbg["oraw00"][:], oev[0 : DH + 1, :, :QB])
        # normalize; stage the denom row at partition 0 (the custom DVE
        # reciprocal misreads inputs at a nonzero base partition)
        drow = npool.tile([1, 2, 512], F32, tag="drow", name="drow")
        nc.vector.tensor_copy(drow[:, :, :QB], o_ps[DH : DH + 1, :, :QB])
        recip = npool.tile([1, 2, 512], F32, tag="recip", name="recip")
        nc.vector.reciprocal_approx_fast(out=recip[:, :, :QB], in_=drow[:, :, :QB])
        bcast = npool.tile([64, 2, 512], F32, tag="bcast", name="bcast")
        nc.gpsimd.partition_broadcast(bcast[:, :, :QB], recip[:, :, :QB])
        if dbg is not None and b == 0 and g == 0:
            nc.sync.dma_start(dbg["recip00"][:], recip[:, :, :QB])
            nc.sync.dma_start(dbg["bcast00"][:], bcast[:, :, :QB])
        for a in range(2):
            nc.vector.tensor_tensor(
                on[g, b][a * 64 : a * 64 + 64, :],
                oev[0:DH, a, :QB],
                bcast[:, a, :QB],
                mybir.AluOpType.mult,
            )

    # ---- emission schedule: attention starts early; remaining projections
    # are interleaved INSIDE sweeps so they fill PE slack without stalling ACT.
    # Emission order is program order: every filler (tile writer) must be
    # emitted before the first sweep that reads its tile.
    from collections import deque
    from functools import partial

    for b in range(NQB):
        emit_qk(wk_sb, kt, 0, b)
    emit_qk(wq_sb, qt, 0, 0)
    nc.sync.dma_start(wo_sb[:], wo[:])

    pend = deque()
    for b in range(1, NQB):
        pend.append((("q", 0, b), partial(emit_qk, wq_sb, qt, 0, b)))
    if PAIRS > 1:
        for b in range(NQB):
            pend.append((("k", 1, b), partial(emit_qk, wk_sb, kt, 1, b)))
        for b in range(NQB):
            pend.append((("q", 1, b), partial(emit_qk, wq_sb, qt, 1, b)))

    def fill_one():
        key, fn = pend.popleft()
        fn()
        emitted.add(key)

    def require(keys):
        while pend and any(k not in emitted for k in keys):
            fill_one()

    def sweep(b, g, **kw):
        keys = [("q", g, b)] if (g, b) != (0, 0) else []
        keys += [("k", g, bb) for bb in range(NQB)] if g > 0 else []
        require(keys)
        attention(b, g, fillers=pend, nfill=kw.pop("nfill", 0), **kw)

    attention(0, 0, with_v=True, fillers=pend, nfill=NQB - 1)
    for b in range(1, NQB):
        sweep(b, 0, nfill=2)
    def out_proj(bb):
        NH = cfg.dim // 512
        for t in range(QB // 128):
            nt = bb * (QB // 128) + t
            for nh in range(NH):
                ps = paQ.tile([P, 512], F32, tag="pa", name="pc")
                for kc in range(M_SLABS):
                    nc.tensor.matmul(
                        ps[:],
                        lhsT=on[kc, bb][:, ts(t, 128)],
                        rhs=wo_sb[:, kc, ts(nh, 512)],
                        start=(kc == 0),
                        stop=(kc == M_SLABS - 1),
                    )
                ot = copool.tile([P, 512], MD, tag="ot", name="ot")
                nc.vector.tensor_copy(ot[:], ps[:])
                nc.sync.dma_start(out[ts(nt, 128), ts(nh, 512)], ot[:])

    if PAIRS > 1:
        for b in range(NQB):
            sweep(b, 1, nfill=1)
    while pend:
        fill_one()
    for b in range(NQB):
        out_proj(b)

    if dbg is not None:
        nc.sync.dma_start(dbg["qt00"][:], qt[0, 0][:])
        nc.sync.dma_start(dbg["kt00"][:], kt[0, 0][:])
        nc.sync.dma_start(dbg["vt0"][:], vt[0][:])
        nc.sync.dma_start(dbg["on00"][:], on[0, 0][:])


def build_program(cfg, num_devices=N_CORES):
    nc = bacc.Bacc("TRN2", target_bir_lowering=False, debug=False, num_devices=num_devices)
    P = 128
    xT = nc.dram_tensor("xT", [P, cfg.kc, cfg.n], cfg.mm_dt, kind="ExternalInput").ap()
    wq = nc.dram_tensor("wq", [P, cfg.kc, cfg.shard], cfg.mm_dt, kind="ExternalInput").ap()
    wk = nc.dram_tensor("wk", [P, cfg.kc, cfg.shard], cfg.mm_dt, kind="ExternalInput").ap()
    wv = nc.dram_tensor("wv", [P, cfg.kc, cfg.vw], cfg.mm_dt, kind="ExternalInput").ap()
    wo = nc.dram_tensor("wo", [P, cfg.shard // 128, cfg.dim], cfg.mm_dt, kind="ExternalInput").ap()
    out = nc.dram_tensor("out", [cfg.n, cfg.dim], cfg.mm_dt, kind="ExternalOutput").ap()
    dbg = None
    if getattr(cfg, "dbg", False):
        QB = cfg.qb
        dbg = {
            "qt00": nc.dram_tensor("qt00", [P, QB], cfg.mm_dt, kind="ExternalOutput").ap(),
            "kt00": nc.dram_tensor("kt00", [P, QB], cfg.mm_dt, kind="ExternalOutput").ap(),
            "vt0": nc.dram_tensor("vt0", [P, cfg.vw], cfg.mm_dt, kind="ExternalOutput").ap(),
            "on00": nc.dram_tensor("on00", [P, QB], cfg.mm_dt, kind="ExternalOutput").ap(),
            "e00": nc.dram_tensor("e00", [P, 2, QB], cfg.mm_dt, kind="ExternalOutput").ap(),
            "oraw00": nc.dram_tensor("oraw00", [DH + 1, 2, QB], F32, kind="ExternalOutput").ap(),
            "recip00": nc.dram_tensor("recip00", [1, 2, QB], F32, kind="ExternalOutput").ap(),
            "bcast00": nc.dram_tensor("bcast00", [64, 2, QB], F32, kind="ExternalOutput").ap(),
        }
    with tile.TileContext(nc) as tc, ExitStack() as ctx:
        build_kernel(tc, ctx, cfg, xT, wq, wk, wv, wo, out, dbg=dbg)
    nc.compile()
    return nc


def shard_inputs(cfg, x, W_qkv, W_out, n_groups):
    """Build per-core input maps. Core c = (batch b, head-group g): c = b*n_groups + g."""
    b_sz = x.shape[0]
    dim, hg, sh = cfg.dim, cfg.hg, cfg.shard
    xTs = []
    for b in range(b_sz):
        xt = np.ascontiguousarray(
            x[b].T.reshape(cfg.kc, 128, cfg.n).transpose(1, 0, 2)
        )
        xTs.append(xt)

    def wlayout(w):  # [dim, C] -> [128, kc, C]
        return np.ascontiguousarray(
            w.reshape(cfg.kc, 128, w.shape[1]).transpose(1, 0, 2)
        )

    in_maps = []
    for b in range(b_sz):
        for g in range(n_groups):
            wq = W_qkv[:, sh * g : sh * (g + 1)]
            wk = W_qkv[:, dim + sh * g : dim + sh * (g + 1)]
            wv_cols = W_qkv[:, 2 * dim + sh * g : 2 * dim + sh * (g + 1)]
            wv = np.zeros((dim, cfg.vw), np.float32)
            for h in range(hg):
                wv[:, h * (DH + 1) : h * (DH + 1) + DH] = wv_cols[:, h * DH : (h + 1) * DH]
            wo = W_out[sh * g : sh * (g + 1), :]
            wo_l = np.ascontiguousarray(
                wo.reshape(sh // 128, 128, dim).transpose(1, 0, 2)
            )
            in_maps.append(
                {
                    "xT": xTs[b].astype(cfg.np_dt),
                    "wq": wlayout(wq).astype(cfg.np_dt),
                    "wk": wlayout(wk).astype(cfg.np_dt),
                    "wv": wlayout(wv).astype(cfg.np_dt),
                    "wo": wo_l.astype(cfg.np_dt),
                }
            )
    return in_maps


_NC_CACHE = {}


def kernel(x, W_qkv, W_out, b_out):
    x = np.asarray(x, np.float32)
    W_qkv = np.asarray(W_qkv, np.float32)
    W_out = np.asarray(W_out, np.float32)
    b_out = np.asarray(b_out, np.float32)
    cfg = FULL
    bsz = x.shape[0]
    n_groups = N_CORES // bsz

    if "nc" not in _NC_CACHE:
        _NC_CACHE["nc"] = build_program(cfg)
    nc = _NC_CACHE["nc"]

    in_maps = shard_inputs(cfg, x, W_qkv, W_out, n_groups)
    res = run_bass_kernel_spmd(nc, in_maps, list(range(N_CORES)))

    out = np.zeros((bsz, cfg.n, cfg.dim), np.float32)
    for b in range(bsz):
        for g in range(n_groups):
            out[b] += res.results[b * n_groups + g]["out"].astype(np.float32)
        out[b] += b_out
    return out


# revision 27
# speedup vs baseline: 1.0584x; 1.0584x over previous
"""Multi-head attention (b=2, n=2048, dim=1024, h=16, fp32) on 8 TRN2 NeuronCores.

Sharding: 2 batches x 4 head-groups (4 heads each). Each core computes, for its
batch element and 4 heads: QKV projection, softmax attention, and a partial
output projection (W_out rows of its heads). Host sums the 4 partials per batch
and adds the bias.

Device layout choices (per core):
  - x arrives pre-transposed (host) as xT [128, 8, 2048]  (p=dim%128, kc=dim//128, n)
  - Q^T/K^T computed as [128, 2, 2048]: pair g holds heads 2g (partitions 0-63)
    and 2g+1 (partitions 64-127); row r of pair g = W_qkv column (g*128+r).
  - S^T = K @ Q^T per head via row-tiled (K=64) matmul pairs; softmax exp on
    ScalarE directly PSUM->SBUF with scale=dim^-0.5 folded in (no max
    subtraction needed: |scores*scale| < ~0.5).
  - V is augmented with a ones column per head ([V_h | 1]) so the PV matmul's
    65th output row accumulates the softmax denominator for free.
  - Normalization: reciprocal_approx_fast (DVE) + partition_broadcast (GPSIMD)
    + one tensor_tensor multiply fused with the PSUM->SBUF evacuation.
  - Matmuls run as float32r (TF32-like single-pass, 4x faster than fp32).
"""

import os
import numpy as np
from contextlib import ExitStack

import concourse.bass as bass
import concourse.mybir as mybir
import concourse.tile as tile
from concourse import bacc
from concourse.bass import ts
from concourse.bass_utils import run_bass_kernel_spmd

F32 = mybir.dt.float32
F32R = mybir.dt.float32r

N_CORES = 8
HEADS = 16
DH = 64  # head dim


class Cfg:
    def __init__(self, n, dim, hg):
        self.n = n                    # sequence length (per core)
        self.dim = dim                # model dim
        self.hg = hg                  # heads per core
        self.kc = dim // 128          # dim chunks of 128
        self.nqb = max(1, n // 512)   # query blocks of 512
        self.qb = min(n, 512)
        self.nkc = n // 128           # key chunks of 128
        self.pairs = hg // 2
        self.shard = hg * DH          # qkv shard columns per section
        self.vw = hg * (DH + 1)       # V columns incl per-head ones col
        self.mm_dt = F32R
        self.np_dt = np.float32
        # S-psum group sizes (banks per exp activate); sum == nkc
        g, rem, sizes = 3, n // 128, []
        while rem > 0:
            sizes.append(min(g, rem))
            rem -= min(g, rem)
        self.groups = sizes


FULL = Cfg(2048, 1024, 4)
# fp16 matmuls: 1 cycle/column streaming + FWL weight loads (fp32r streams at
# ~2 cyc/col). Measured end-to-end relative error 4.1e-4 vs the fp32
# reference (fp32r variant: 1.8e-4 at ~15% more time). Set ATTN_MM_DT=fp32r
# to switch back.
if os.environ.get("ATTN_MM_DT", "fp16") == "fp16":
    FULL.mm_dt = mybir.dt.float16
    FULL.np_dt = np.float16


def build_kernel(tc, ctx, cfg, xT, wq, wk, wv, wo, out, dbg=None):
    nc = tc.nc
    P = 128
    KC, NQB, QB, NKC, PAIRS = cfg.kc, cfg.nqb, cfg.qb, cfg.nkc, cfg.pairs
    MD = cfg.mm_dt
    SCALE = cfg.dim ** -0.5
    M_SLABS = cfg.shard // 128  # = PAIRS

    def r(ap):
        return ap

    wpool = ctx.enter_context(tc.tile_pool(name="w", bufs=1))
    wq_sb = wpool.tile([P, KC, cfg.shard], MD, tag="wq", name="wq_sb")
    wk_sb = wpool.tile([P, KC, cfg.shard], MD, tag="wk", name="wk_sb")
    wv_sb = wpool.tile([P, KC, cfg.vw], MD, tag="wv", name="wv_sb")
    wo_sb = wpool.tile([P, M_SLABS, cfg.dim], MD, tag="wo", name="wo_sb")

    per = ctx.enter_context(tc.tile_pool(name="per", bufs=1))
    qt = {}  # (pair, nqb) -> [128, QB]
    kt = {}
    vt = {}  # nt -> [128, vw]
    on = {}  # (slab, nqb) -> [128, QB]  normalized O^T for out-proj lhsT
    for g in range(PAIRS):
        for b in range(NQB):
            qt[g, b] = per.tile([P, QB], MD, tag=f"qt{g}_{b}", name=f"qt{g}_{b}")
            kt[g, b] = per.tile([P, QB], MD, tag=f"kt{g}_{b}", name=f"kt{g}_{b}")
            on[g, b] = per.tile([P, QB], MD, tag=f"on{g}_{b}", name=f"on{g}_{b}")
    for t in range(NKC):
        vt[t] = per.tile([P, cfg.vw], MD, tag=f"v{t}", name=f"v{t}")

    xpool = ctx.enter_context(tc.tile_pool(name="x", bufs=1))
    paQ = ctx.enter_context(tc.tile_pool(name="paQ", bufs=2, space="PSUM"))
    psS = ctx.enter_context(tc.tile_pool(name="psS", bufs=2, space="PSUM"))
    psO = ctx.enter_context(tc.tile_pool(name="psO", bufs=1, space="PSUM"))
    epool = ctx.enter_context(tc.tile_pool(name="e", bufs=6))
    npool = ctx.enter_context(tc.tile_pool(name="nrm", bufs=3))
    copool = ctx.enter_context(tc.tile_pool(name="co", bufs=6))

    xts = {}
    for b in range(NQB):
        xts[b] = xpool.tile([P, KC, QB], MD, tag=f"xt{b}", name=f"xt{b}")
    h = KC // 2
    nc.sync.dma_start(wk_sb[:, :h], wk[:, :h])
    nc.sync.dma_start(xts[0][:, :h], xT[:, :h, ts(0, QB)])
    nc.sync.dma_start(wk_sb[:, h:], wk[:, h:])
    nc.sync.dma_start(xts[0][:, h:], xT[:, h:, ts(0, QB)])
    nc.sync.dma_start(wq_sb[:], wq[:])
    for b in range(1, NQB):
        nc.sync.dma_start(xts[b][:], xT[:, :, ts(b, QB)])
    nc.sync.dma_start(wv_sb[:], wv[:])

    def emit_qk(w_sb, dst, g, b):
        ps = paQ.tile([P, 512], F32, tag="pa", name="pa")
        for kc in range(KC):
            nc.tensor.matmul(
                ps[:, :QB],
                lhsT=w_sb[:, kc, ts(g, 128)],
                rhs=xts[b][:, kc, :],
                start=(kc == 0),
                stop=(kc == KC - 1),
            )
        nc.vector.tensor_copy(dst[g, b][:], ps[:, :QB])

    def emit_v(nt):
        vb, t = divmod(nt, QB // 128)
        ps = paQ.tile([P, 512], F32, tag="pa", name="pa")
        for kc in range(KC):
            nc.tensor.matmul(
                ps[:, : cfg.vw],
                lhsT=xts[vb][:, kc, ts(t, 128)],
                rhs=wv_sb[:, kc, :],
                start=(kc == 0),
                stop=(kc == KC - 1),
            )
        nc.vector.tensor_copy(vt[nt][:], ps[:, : cfg.vw])
        v4 = vt[nt][:].rearrange("p (h e) -> p h e", e=DH + 1)
        ones_ap = v4[:, :, DH : DH + 1]
        if mybir.dt.size(MD) == 4:
            ones_ap = ones_ap.bitcast(F32)
        nc.vector.memset(ones_ap, 1.0)

    emitted = set()

    def attention(b, g, with_v=False, fillers=None, nfill=0):
        o_ps = psO.tile([P, 2, 512], F32, tag="o", name="o_ps")
        done = 0
        for c in range(NKC):
            if with_v:
                emit_v(c)
            if fillers and done < nfill and c in (5, 9, 13):
                key, fn = fillers.popleft()
                fn()
                emitted.add(key)
                done += 1
            s_ps = psS.tile([P, 2, 512], F32, tag="s", name="s_ps")
            for a in range(2):
                lo = a * 64
                nc.tensor.matmul(
                    s_ps[:, a, :QB],
                    lhsT=kt[g, c * 128 // QB][lo : lo + 64, ts(c % (QB // 128), 128)],
                    rhs=qt[g, b][lo : lo + 64, :],
                    start=True,
                    stop=True,
                )
            e_t = epool.tile([P, 2, 512], MD, tag="e", name="e_t")
            nc.scalar.activation(
                e_t[:, :, :QB],
                s_ps[:, :, :QB],
                mybir.ActivationFunctionType.Exp,
                scale=SCALE,
            )
            if dbg is not None and b == 0 and g == 0 and c == 0:
                nc.sync.dma_start(dbg["e00"][:], e_t[:, :, :QB])
            v4 = vt[c][:].rearrange("p (h e) -> p h e", e=DH + 1)
            for a in range(2):
                h = 2 * g + a
                nc.tensor.matmul(
                    o_ps[0 : DH + 1, a, :QB],
                    lhsT=v4[:, h, :],
                    rhs=e_t[:, a, :QB],
                    start=(c == 0),
                    stop=(c == NKC - 1),
                )
        # one-shot evacuation frees the O psum banks immediately
        oev = npool.tile([P, 2, 512], F32, tag="oev", name="oev")
        nc.vector.tensor_copy(oev[0 : DH + 1, :, :QB], o_ps[0 : DH + 1, :, :QB])
        if dbg is not None and b == 0 and g == 0:
            nc.sync.dma_start(dbg["oraw00"][:], oev[0 : DH + 1, :, :QB])
        # normalize; stage the denom row at partition 0 (the custom DVE
        # reciprocal misreads inputs at a nonzero base partition)
        drow = npool.tile([1, 2, 512], F32, tag="drow", name="drow")
        nc.vector.tensor_copy(drow[:, :, :QB], o_ps[DH : DH + 1, :, :QB])
        recip = npool.tile([1, 2, 512], F32, tag="recip", name="recip")
        nc.vector.reciprocal_approx_fast(out=recip[:, :, :QB], in_=drow[:, :, :QB])
        bcast = npool.tile([64, 2, 512], F32, tag="bcast", name="bcast")
        nc.gpsimd.partition_broadcast(bcast[:, :, :QB], recip[:, :, :QB])
        if dbg is not None and b == 0 and g == 0:
            nc.sync.dma_start(dbg["recip00"][:], recip[:, :, :QB])
            nc.sync.dma_start(dbg["bcast00"][:], bcast[:, :, :QB])
        for a in range(2):
            nc.vector.tensor_tensor(
                on[g, b][a * 64 : a * 64 + 64, :],
                oev[0:DH, a, :QB],
                bcast[:, a, :QB],
                mybir.AluOpType.mult,
            )

    # ---- emission schedule: attention starts early; remaining projections
    # are interleaved INSIDE sweeps so they fill PE slack without stalling ACT.
    # Emission order is program order: every filler (tile writer) must be
    # emitted before the first sweep that reads its tile.
    from collections import deque
    from functools import partial

    for b in range(NQB):
        emit_qk(wk_sb, kt, 0, b)
    emit_qk(wq_sb, qt, 0, 0)
    nc.sync.dma_start(wo_sb[:], wo[:])

    pend = deque()
    for b in range(1, NQB):
        pend.append((("q", 0, b), partial(emit_qk, wq_sb, qt, 0, b)))
    if PAIRS > 1:
        for b in range(NQB):
            pend.append((("k", 1, b), partial(emit_qk, wk_sb, kt, 1, b)))
        for b in range(NQB):
            pend.append((("q", 1, b), partial(emit_qk, wq_sb, qt, 1, b)))

    def fill_one():
        key, fn = pend.popleft()
        fn()
        emitted.add(key)

    def require(keys):
        while pend and any(k not in emitted for k in keys):
            fill_one()

    def sweep(b, g, **kw):
        keys = [("q", g, b)] if (g, b) != (0, 0) else []
        keys += [("k", g, bb) for bb in range(NQB)] if g > 0 else []
        require(keys)
        attention(b, g, fillers=pend, nfill=kw.pop("nfill", 0), **kw)

    attention(0, 0, with_v=True, fillers=pend, nfill=NQB - 1)
    for b in range(1, NQB):
        sweep(b, 0, nfill=2)
    def out_proj(bb):
        NH = cfg.dim // 512
        for t in range(QB // 128):
            nt = bb * (QB // 128) + t
            for nh in range(NH):
                ps = paQ.tile([P, 512], F32, tag="pa", name="pc")
                for kc in range(M_SLABS):
                    nc.tensor.matmul(
                        ps[:],
                        lhsT=on[kc, bb][:, ts(t, 128)],
                        rhs=wo_sb[:, kc, ts(nh, 512)],
                        start=(kc == 0),
                        stop=(kc == M_SLABS - 1),
                    )
                ot = copool.tile([P, 512], MD, tag="ot", name="ot")
                nc.vector.tensor_copy(ot[:], ps[:])
                nc.sync.dma_start(out[ts(nt, 128), ts(nh, 512)], ot[:])

    if PAIRS > 1:
        for b in range(NQB):
            sweep(b, 1, nfill=1)
    while pend:
        fill_one()
    for b in range(NQB):
        out_proj(b)

    if dbg is not None:
        nc.sync.dma_start(dbg["qt00"][:], qt[0, 0][:])
        nc.sync.dma_start(dbg["kt00"][:], kt[0, 0][:])
        nc.sync.dma_start(dbg["vt0"][:], vt[0][:])
        nc.sync.dma_start(dbg["on00"][:], on[0, 0][:])


def build_program(cfg, num_devices=N_CORES):
    nc = bacc.Bacc("TRN2", target_bir_lowering=False, debug=False, num_devices=num_devices)
    P = 128
    xT = nc.dram_tensor("xT", [P, cfg.kc, cfg.n], cfg.mm_dt, kind="ExternalInput").ap()
    wq = nc.dram_tensor("wq", [P, cfg.kc, cfg.shard], cfg.mm_dt, kind="ExternalInput").ap()
    wk = nc.dram_tensor("wk", [P, cfg.kc, cfg.shard], cfg.mm_dt, kind="ExternalInput").ap()
    wv = nc.dram_tensor("wv", [P, cfg.kc, cfg.vw], cfg.mm_dt, kind="ExternalInput").ap()
    wo = nc.dram_tensor("wo", [P, cfg.shard // 128, cfg.dim], cfg.mm_dt, kind="ExternalInput").ap()
    out = nc.dram_tensor("out", [cfg.n, cfg.dim], cfg.mm_dt, kind="ExternalOutput").ap()
    dbg = None
    if getattr(cfg, "dbg", False):
        QB = cfg.qb
        dbg = {
            "qt00": nc.dram_tensor("qt00", [P, QB], cfg.mm_dt, kind="ExternalOutput").ap(),
            "kt00": nc.dram_tensor("kt00", [P, QB], cfg.mm_dt, kind="ExternalOutput").ap(),
            "vt0": nc.dram_tensor("vt0", [P, cfg.vw], cfg.mm_dt, kind="ExternalOutput").ap(),
            "on00": nc.dram_tensor("on00", [P, QB], cfg.mm_dt, kind="ExternalOutput").ap(),
            "e00": nc.dram_tensor("e00", [P, 2, QB], cfg.mm_dt, kind="ExternalOutput").ap(),
            "oraw00": nc.dram_tensor("oraw00", [DH + 1, 2, QB], F32, kind="ExternalOutput").ap(),
            "recip00": nc.dram_tensor("recip00", [1, 2, QB], F32, kind="ExternalOutput").ap(),
            "bcast00": nc.dram_tensor("bcast00", [64, 2, QB], F32, kind="ExternalOutput").ap(),
        }
    with tile.TileContext(nc) as tc, ExitStack() as ctx:
        build_kernel(tc, ctx, cfg, xT, wq, wk, wv, wo, out, dbg=dbg)
    nc.compile()
    return nc


def shard_inputs(cfg, x, W_qkv, W_out, n_groups):
    """Build per-core input maps. Core c = (batch b, head-group g): c = b*n_groups + g."""
    b_sz = x.shape[0]
    dim, hg, sh = cfg.dim, cfg.hg, cfg.shard
    xTs = []
    for b in range(b_sz):
        xt = np.ascontiguousarray(
            x[b].T.reshape(cfg.kc, 128, cfg.n).transpose(1, 0, 2)
        )
        xTs.append(xt)

    def wlayout(w):  # [dim, C] -> [128, kc, C]
        return np.ascontiguousarray(
            w.reshape(cfg.kc, 128, w.shape[1]).transpose(1, 0, 2)
        )

    in_maps = []
    for b in range(b_sz):
        for g in range(n_groups):
            wq = W_qkv[:, sh * g : sh * (g + 1)]
            wk = W_qkv[:, dim + sh * g : dim + sh * (g + 1)]
            wv_cols = W_qkv[:, 2 * dim + sh * g : 2 * dim + sh * (g + 1)]
            wv = np.zeros((dim, cfg.vw), np.float32)
            for h in range(hg):
                wv[:, h * (DH + 1) : h * (DH + 1) + DH] = wv_cols[:, h * DH : (h + 1) * DH]
            wo = W_out[sh * g : sh * (g + 1), :]
            wo_l = np.ascontiguousarray(
                wo.reshape(sh // 128, 128, dim).transpose(1, 0, 2)
            )
            in_maps.append(
                {
                    "xT": xTs[b].astype(cfg.np_dt),
                    "wq": wlayout(wq).astype(cfg.np_dt),
                    "wk": wlayout(wk).astype(cfg.np_dt),
                    "wv": wlayout(wv).astype(cfg.np_dt),
                    "wo": wo_l.astype(cfg.np_dt),
                }
            )
    return in_maps


_NC_CACHE = {}


def kernel(x, W_qkv, W_out, b_out):
    x = np.asarray(x, np.float32)
    W_qkv = np.asarray(W_qkv, np.float32)
    W_out = np.asarray(W_out, np.float32)
    b_out = np.asarray(b_out, np.float32)
    cfg = FULL
    bsz = x.shape[0]
    n_groups = N_CORES // bsz

    if "nc" not in _NC_CACHE:
        _NC_CACHE["nc"] = build_program(cfg)
    nc = _NC_CACHE["nc"]

    in_maps = shard_inputs(cfg, x, W_qkv, W_out, n_groups)
    res = run_bass_kernel_spmd(nc, in_maps, list(range(N_CORES)))

    out = np.zeros((bsz, cfg.n, cfg.dim), np.float32)
    for b in range(bsz):
        for g in range(n_groups):
            out[b] += res.results[b * n_groups + g]["out"].astype(np.float32)
        out[b] += b_out
    return out


# revision 28
# speedup vs baseline: 1.0591x; 1.0007x over previous
"""Multi-head attention (b=2, n=2048, dim=1024, h=16, fp32) on 8 TRN2 NeuronCores.

Sharding: 2 batches x 4 head-groups (4 heads each). Each core computes, for its
batch element and 4 heads: QKV projection, softmax attention, and a partial
output projection (W_out rows of its heads). Host sums the 4 partials per batch
and adds the bias.

Device layout choices (per core):
  - x arrives pre-transposed (host) as xT [128, 8, 2048]  (p=dim%128, kc=dim//128, n)
  - Q^T/K^T computed as [128, 2, 2048]: pair g holds heads 2g (partitions 0-63)
    and 2g+1 (partitions 64-127); row r of pair g = W_qkv column (g*128+r).
  - S^T = K @ Q^T per head via row-tiled (K=64) matmul pairs; softmax exp on
    ScalarE directly PSUM->SBUF with scale=dim^-0.5 folded in (no max
    subtraction needed: |scores*scale| < ~0.5).
  - V is augmented with a ones column per head ([V_h | 1]) so the PV matmul's
    65th output row accumulates the softmax denominator for free.
  - Normalization: reciprocal_approx_fast (DVE) + partition_broadcast (GPSIMD)
    + one tensor_tensor multiply fused with the PSUM->SBUF evacuation.
  - Matmuls run as float32r (TF32-like single-pass, 4x faster than fp32).
"""

import os
import numpy as np
from contextlib import ExitStack

import concourse.bass as bass
import concourse.mybir as mybir
import concourse.tile as tile
from concourse import bacc
from concourse.bass import ts
from concourse.bass_utils import run_bass_kernel_spmd

F32 = mybir.dt.float32
F32R = mybir.dt.float32r

N_CORES = 8
HEADS = 16
DH = 64  # head dim


class Cfg:
    def __init__(self, n, dim, hg):
        self.n = n                    # sequence length (per core)
        self.dim = dim                # model dim
        self.hg = hg                  # heads per core
        self.kc = dim // 128          # dim chunks of 128
        self.nqb = max(1, n // 512)   # query blocks of 512
        self.qb = min(n, 512)
        self.nkc = n // 128           # key chunks of 128
        self.pairs = hg // 2
        self.shard = hg * DH          # qkv shard columns per section
        self.vw = hg * (DH + 1)       # V columns incl per-head ones col
        self.mm_dt = F32R
        self.np_dt = np.float32
        # S-psum group sizes (banks per exp activate); sum == nkc
        g, rem, sizes = 3, n // 128, []
        while rem > 0:
            sizes.append(min(g, rem))
            rem -= min(g, rem)
        self.groups = sizes


FULL = Cfg(2048, 1024, 4)
# fp16 matmuls: 1 cycle/column streaming + FWL weight loads (fp32r streams at
# ~2 cyc/col). Measured end-to-end relative error 4.1e-4 vs the fp32
# reference (fp32r variant: 1.8e-4 at ~15% more time). Set ATTN_MM_DT=fp32r
# to switch back.
if os.environ.get("ATTN_MM_DT", "fp16") == "fp16":
    FULL.mm_dt = mybir.dt.float16
    FULL.np_dt = np.float16


def build_kernel(tc, ctx, cfg, xT, wq, wk, wv, wo, out, dbg=None):
    nc = tc.nc
    P = 128
    KC, NQB, QB, NKC, PAIRS = cfg.kc, cfg.nqb, cfg.qb, cfg.nkc, cfg.pairs
    MD = cfg.mm_dt
    SCALE = cfg.dim ** -0.5
    M_SLABS = cfg.shard // 128  # = PAIRS

    def r(ap):
        return ap

    wpool = ctx.enter_context(tc.tile_pool(name="w", bufs=1))
    wq_sb = wpool.tile([P, KC, cfg.shard], MD, tag="wq", name="wq_sb")
    wk_sb = wpool.tile([P, KC, cfg.shard], MD, tag="wk", name="wk_sb")
    wv_sb = wpool.tile([P, KC, cfg.vw], MD, tag="wv", name="wv_sb")
    wo_sb = wpool.tile([P, M_SLABS, cfg.dim], MD, tag="wo", name="wo_sb")

    per = ctx.enter_context(tc.tile_pool(name="per", bufs=1))
    qt = {}  # (pair, nqb) -> [128, QB]
    kt = {}
    vt = {}  # nt -> [128, vw]
    on = {}  # (slab, nqb) -> [128, QB]  normalized O^T for out-proj lhsT
    for g in range(PAIRS):
        for b in range(NQB):
            qt[g, b] = per.tile([P, QB], MD, tag=f"qt{g}_{b}", name=f"qt{g}_{b}")
            kt[g, b] = per.tile([P, QB], MD, tag=f"kt{g}_{b}", name=f"kt{g}_{b}")
            on[g, b] = per.tile([P, QB], MD, tag=f"on{g}_{b}", name=f"on{g}_{b}")
    for t in range(NKC):
        vt[t] = per.tile([P, cfg.vw], MD, tag=f"v{t}", name=f"v{t}")

    xpool = ctx.enter_context(tc.tile_pool(name="x", bufs=1))
    paQ = ctx.enter_context(tc.tile_pool(name="paQ", bufs=2, space="PSUM"))
    psS = ctx.enter_context(tc.tile_pool(name="psS", bufs=2, space="PSUM"))
    psO = ctx.enter_context(tc.tile_pool(name="psO", bufs=1, space="PSUM"))
    epool = ctx.enter_context(tc.tile_pool(name="e", bufs=10))
    npool = ctx.enter_context(tc.tile_pool(name="nrm", bufs=3))
    copool = ctx.enter_context(tc.tile_pool(name="co", bufs=6))

    xts = {}
    for b in range(NQB):
        xts[b] = xpool.tile([P, KC, QB], MD, tag=f"xt{b}", name=f"xt{b}")
    h = KC // 2
    nc.sync.dma_start(wk_sb[:, :h], wk[:, :h])
    nc.sync.dma_start(xts[0][:, :h], xT[:, :h, ts(0, QB)])
    nc.sync.dma_start(wk_sb[:, h:], wk[:, h:])
    nc.sync.dma_start(xts[0][:, h:], xT[:, h:, ts(0, QB)])
    nc.sync.dma_start(wq_sb[:], wq[:])
    for b in range(1, NQB):
        nc.sync.dma_start(xts[b][:], xT[:, :, ts(b, QB)])
    nc.sync.dma_start(wv_sb[:], wv[:])

    def emit_qk(w_sb, dst, g, b):
        ps = paQ.tile([P, 512], F32, tag="pa", name="pa")
        for kc in range(KC):
            nc.tensor.matmul(
                ps[:, :QB],
                lhsT=w_sb[:, kc, ts(g, 128)],
                rhs=xts[b][:, kc, :],
                start=(kc == 0),
                stop=(kc == KC - 1),
            )
        nc.vector.tensor_copy(dst[g, b][:], ps[:, :QB])

    def emit_v(nt):
        vb, t = divmod(nt, QB // 128)
        ps = paQ.tile([P, 512], F32, tag="pa", name="pa")
        for kc in range(KC):
            nc.tensor.matmul(
                ps[:, : cfg.vw],
                lhsT=xts[vb][:, kc, ts(t, 128)],
                rhs=wv_sb[:, kc, :],
                start=(kc == 0),
                stop=(kc == KC - 1),
            )
        nc.vector.tensor_copy(vt[nt][:], ps[:, : cfg.vw])
        v4 = vt[nt][:].rearrange("p (h e) -> p h e", e=DH + 1)
        ones_ap = v4[:, :, DH : DH + 1]
        if mybir.dt.size(MD) == 4:
            ones_ap = ones_ap.bitcast(F32)
        nc.vector.memset(ones_ap, 1.0)

    emitted = set()

    def attention(b, g, with_v=False, fillers=None, nfill=0):
        o_ps = psO.tile([P, 2, 512], F32, tag="o", name="o_ps")
        done = 0
        for c in range(NKC):
            if with_v:
                emit_v(c)
            if fillers and done < nfill and c in (5, 9, 13):
                key, fn = fillers.popleft()
                fn()
                emitted.add(key)
                done += 1
            s_ps = psS.tile([P, 2, 512], F32, tag="s", name="s_ps")
            for a in range(2):
                lo = a * 64
                nc.tensor.matmul(
                    s_ps[:, a, :QB],
                    lhsT=kt[g, c * 128 // QB][lo : lo + 64, ts(c % (QB // 128), 128)],
                    rhs=qt[g, b][lo : lo + 64, :],
                    start=True,
                    stop=True,
                )
            e_t = epool.tile([P, 2, 512], MD, tag="e", name="e_t")
            nc.scalar.activation(
                e_t[:, :, :QB],
                s_ps[:, :, :QB],
                mybir.ActivationFunctionType.Exp,
                scale=SCALE,
            )
            if dbg is not None and b == 0 and g == 0 and c == 0:
                nc.sync.dma_start(dbg["e00"][:], e_t[:, :, :QB])
            v4 = vt[c][:].rearrange("p (h e) -> p h e", e=DH + 1)
            for a in range(2):
                h = 2 * g + a
                nc.tensor.matmul(
                    o_ps[0 : DH + 1, a, :QB],
                    lhsT=v4[:, h, :],
                    rhs=e_t[:, a, :QB],
                    start=(c == 0),
                    stop=(c == NKC - 1),
                )
        # one-shot evacuation frees the O psum banks immediately
        oev = npool.tile([P, 2, 512], F32, tag="oev", name="oev")
        nc.vector.tensor_copy(oev[0 : DH + 1, :, :QB], o_ps[0 : DH + 1, :, :QB])
        if dbg is not None and b == 0 and g == 0:
            nc.sync.dma_start(dbg["oraw00"][:], oev[0 : DH + 1, :, :QB])
        # normalize; stage the denom row at partition 0 (the custom DVE
        # reciprocal misreads inputs at a nonzero base partition)
        drow = npool.tile([1, 2, 512], F32, tag="drow", name="drow")
        nc.vector.tensor_copy(drow[:, :, :QB], o_ps[DH : DH + 1, :, :QB])
        recip = npool.tile([1, 2, 512], F32, tag="recip", name="recip")
        nc.vector.reciprocal_approx_fast(out=recip[:, :, :QB], in_=drow[:, :, :QB])
        bcast = npool.tile([64, 2, 512], F32, tag="bcast", name="bcast")
        nc.gpsimd.partition_broadcast(bcast[:, :, :QB], recip[:, :, :QB])
        if dbg is not None and b == 0 and g == 0:
            nc.sync.dma_start(dbg["recip00"][:], recip[:, :, :QB])
            nc.sync.dma_start(dbg["bcast00"][:], bcast[:, :, :QB])
        for a in range(2):
            nc.vector.tensor_tensor(
                on[g, b][a * 64 : a * 64 + 64, :],
                oev[0:DH, a, :QB],
                bcast[:, a, :QB],
                mybir.AluOpType.mult,
            )

    # ---- emission schedule: attention starts early; remaining projections
    # are interleaved INSIDE sweeps so they fill PE slack without stalling ACT.
    # Emission order is program order: every filler (tile writer) must be
    # emitted before the first sweep that reads its tile.
    from collections import deque
    from functools import partial

    for b in range(NQB):
        emit_qk(wk_sb, kt, 0, b)
    emit_qk(wq_sb, qt, 0, 0)
    nc.sync.dma_start(wo_sb[:], wo[:])

    pend = deque()
    for b in range(1, NQB):
        pend.append((("q", 0, b), partial(emit_qk, wq_sb, qt, 0, b)))
    if PAIRS > 1:
        for b in range(NQB):
            pend.append((("k", 1, b), partial(emit_qk, wk_sb, kt, 1, b)))
        for b in range(NQB):
            pend.append((("q", 1, b), partial(emit_qk, wq_sb, qt, 1, b)))

    def fill_one():
        key, fn = pend.popleft()
        fn()
        emitted.add(key)

    def require(keys):
        while pend and any(k not in emitted for k in keys):
            fill_one()

    def sweep(b, g, **kw):
        keys = [("q", g, b)] if (g, b) != (0, 0) else []
        keys += [("k", g, bb) for bb in range(NQB)] if g > 0 else []
        require(keys)
        attention(b, g, fillers=pend, nfill=kw.pop("nfill", 0), **kw)

    attention(0, 0, with_v=True, fillers=pend, nfill=NQB - 1)
    for b in range(1, NQB):
        sweep(b, 0, nfill=2)
    def out_proj(bb):
        NH = cfg.dim // 512
        for t in range(QB // 128):
            nt = bb * (QB // 128) + t
            for nh in range(NH):
                ps = paQ.tile([P, 512], F32, tag="pa", name="pc")
                for kc in range(M_SLABS):
                    nc.tensor.matmul(
                        ps[:],
                        lhsT=on[kc, bb][:, ts(t, 128)],
                        rhs=wo_sb[:, kc, ts(nh, 512)],
                        start=(kc == 0),
                        stop=(kc == M_SLABS - 1),
                    )
                ot = copool.tile([P, 512], MD, tag="ot", name="ot")
                nc.vector.tensor_copy(ot[:], ps[:])
                nc.sync.dma_start(out[ts(nt, 128), ts(nh, 512)], ot[:])

    if PAIRS > 1:
        for b in range(NQB):
            sweep(b, 1, nfill=1)
    while pend:
        fill_one()
    for b in range(NQB):
        out_proj(b)

    if dbg is not None:
        nc.sync.dma_start(dbg["qt00"][:], qt[0, 0][:])
        nc.sync.dma_start(dbg["kt00"][:], kt[0, 0][:])
        nc.sync.dma_start(dbg["vt0"][:], vt[0][:])
        nc.sync.dma_start(dbg["on00"][:], on[0, 0][:])


def build_program(cfg, num_devices=N_CORES):
    nc = bacc.Bacc("TRN2", target_bir_lowering=False, debug=False, num_devices=num_devices)
    P = 128
    xT = nc.dram_tensor("xT", [P, cfg.kc, cfg.n], cfg.mm_dt, kind="ExternalInput").ap()
    wq = nc.dram_tensor("wq", [P, cfg.kc, cfg.shard], cfg.mm_dt, kind="ExternalInput").ap()
    wk = nc.dram_tensor("wk", [P, cfg.kc, cfg.shard], cfg.mm_dt, kind="ExternalInput").ap()
    wv = nc.dram_tensor("wv", [P, cfg.kc, cfg.vw], cfg.mm_dt, kind="ExternalInput").ap()
    wo = nc.dram_tensor("wo", [P, cfg.shard // 128, cfg.dim], cfg.mm_dt, kind="ExternalInput").ap()
    out = nc.dram_tensor("out", [cfg.n, cfg.dim], cfg.mm_dt, kind="ExternalOutput").ap()
    dbg = None
    if getattr(cfg, "dbg", False):
        QB = cfg.qb
        dbg = {
            "qt00": nc.dram_tensor("qt00", [P, QB], cfg.mm_dt, kind="ExternalOutput").ap(),
            "kt00": nc.dram_tensor("kt00", [P, QB], cfg.mm_dt, kind="ExternalOutput").ap(),
            "vt0": nc.dram_tensor("vt0", [P, cfg.vw], cfg.mm_dt, kind="ExternalOutput").ap(),
            "on00": nc.dram_tensor("on00", [P, QB], cfg.mm_dt, kind="ExternalOutput").ap(),
            "e00": nc.dram_tensor("e00", [P, 2, QB], cfg.mm_dt, kind="ExternalOutput").ap(),
            "oraw00": nc.dram_tensor("oraw00", [DH + 1, 2, QB], F32, kind="ExternalOutput").ap(),
            "recip00": nc.dram_tensor("recip00", [1, 2, QB], F32, kind="ExternalOutput").ap(),
            "bcast00": nc.dram_tensor("bcast00", [64, 2, QB], F32, kind="ExternalOutput").ap(),
        }
    with tile.TileContext(nc) as tc, ExitStack() as ctx:
        build_kernel(tc, ctx, cfg, xT, wq, wk, wv, wo, out, dbg=dbg)
    nc.compile()
    return nc


def shard_inputs(cfg, x, W_qkv, W_out, n_groups):
    """Build per-core input maps. Core c = (batch b, head-group g): c = b*n_groups + g."""
    b_sz = x.shape[0]
    dim, hg, sh = cfg.dim, cfg.hg, cfg.shard
    xTs = []
    for b in range(b_sz):
        xt = np.ascontiguousarray(
            x[b].T.reshape(cfg.kc, 128, cfg.n).transpose(1, 0, 2)
        )
        xTs.append(xt)

    def wlayout(w):  # [dim, C] -> [128, kc, C]
        return np.ascontiguousarray(
            w.reshape(cfg.kc, 128, w.shape[1]).transpose(1, 0, 2)
        )

    in_maps = []
    for b in range(b_sz):
        for g in range(n_groups):
            wq = W_qkv[:, sh * g : sh * (g + 1)]
            wk = W_qkv[:, dim + sh * g : dim + sh * (g + 1)]
            wv_cols = W_qkv[:, 2 * dim + sh * g : 2 * dim + sh * (g + 1)]
            wv = np.zeros((dim, cfg.vw), np.float32)
            for h in range(hg):
                wv[:, h * (DH + 1) : h * (DH + 1) + DH] = wv_cols[:, h * DH : (h + 1) * DH]
            wo = W_out[sh * g : sh * (g + 1), :]
            wo_l = np.ascontiguousarray(
                wo.reshape(sh // 128, 128, dim).transpose(1, 0, 2)
            )
            in_maps.append(
                {
                    "xT": xTs[b].astype(cfg.np_dt),
                    "wq": wlayout(wq).astype(cfg.np_dt),
                    "wk": wlayout(wk).astype(cfg.np_dt),
                    "wv": wlayout(wv).astype(cfg.np_dt),
                    "wo": wo_l.astype(cfg.np_dt),
                }
            )
    return in_maps


_NC_CACHE = {}


def kernel(x, W_qkv, W_out, b_out):
    x = np.asarray(x, np.float32)
    W_qkv = np.asarray(W_qkv, np.float32)
    W_out = np.asarray(W_out, np.float32)
    b_out = np.asarray(b_out, np.float32)
    cfg = FULL
    bsz = x.shape[0]
    n_groups = N_CORES // bsz

    if "nc" not in _NC_CACHE:
        _NC_CACHE["nc"] = build_program(cfg)
    nc = _NC_CACHE["nc"]

    in_maps = shard_inputs(cfg, x, W_qkv, W_out, n_groups)
    res = run_bass_kernel_spmd(nc, in_maps, list(range(N_CORES)))

    out = np.zeros((bsz, cfg.n, cfg.dim), np.float32)
    for b in range(bsz):
        for g in range(n_groups):
            out[b] += res.results[b * n_groups + g]["out"].astype(np.float32)
        out[b] += b_out
    return out


# revision 29
# speedup vs baseline: 1.0636x; 1.0042x over previous
"""Multi-head attention (b=2, n=2048, dim=1024, h=16, fp32) on 8 TRN2 NeuronCores.

Sharding: 2 batches x 4 head-groups (4 heads each). Each core computes, for its
batch element and 4 heads: QKV projection, softmax attention, and a partial
output projection (W_out rows of its heads). Host sums the 4 partials per batch
and adds the bias.

Device layout choices (per core):
  - x arrives pre-transposed (host) as xT [128, 8, 2048]  (p=dim%128, kc=dim//128, n)
  - Q^T/K^T computed as [128, 2, 2048]: pair g holds heads 2g (partitions 0-63)
    and 2g+1 (partitions 64-127); row r of pair g = W_qkv column (g*128+r).
  - S^T = K @ Q^T per head via row-tiled (K=64) matmul pairs; softmax exp on
    ScalarE directly PSUM->SBUF with scale=dim^-0.5 folded in (no max
    subtraction needed: |scores*scale| < ~0.5).
  - V is augmented with a ones column per head ([V_h | 1]) so the PV matmul's
    65th output row accumulates the softmax denominator for free.
  - Normalization: reciprocal_approx_fast (DVE) + partition_broadcast (GPSIMD)
    + one tensor_tensor multiply fused with the PSUM->SBUF evacuation.
  - Matmuls run as float32r (TF32-like single-pass, 4x faster than fp32).
"""

import os
import numpy as np
from contextlib import ExitStack

import concourse.bass as bass
import concourse.mybir as mybir
import concourse.tile as tile
from concourse import bacc
from concourse.bass import ts
from concourse.bass_utils import run_bass_kernel_spmd

F32 = mybir.dt.float32
F32R = mybir.dt.float32r

N_CORES = 8
HEADS = 16
DH = 64  # head dim


class Cfg:
    def __init__(self, n, dim, hg):
        self.n = n                    # sequence length (per core)
        self.dim = dim                # model dim
        self.hg = hg                  # heads per core
        self.kc = dim // 128          # dim chunks of 128
        self.nqb = max(1, n // 512)   # query blocks of 512
        self.qb = min(n, 512)
        self.nkc = n // 128           # key chunks of 128
        self.pairs = hg // 2
        self.shard = hg * DH          # qkv shard columns per section
        self.vw = hg * (DH + 1)       # V columns incl per-head ones col
        self.mm_dt = F32R
        self.np_dt = np.float32
        # S-psum group sizes (banks per exp activate); sum == nkc
        g, rem, sizes = 3, n // 128, []
        while rem > 0:
            sizes.append(min(g, rem))
            rem -= min(g, rem)
        self.groups = sizes


FULL = Cfg(2048, 1024, 4)
# fp16 matmuls: 1 cycle/column streaming + FWL weight loads (fp32r streams at
# ~2 cyc/col). Measured end-to-end relative error 4.1e-4 vs the fp32
# reference (fp32r variant: 1.8e-4 at ~15% more time). Set ATTN_MM_DT=fp32r
# to switch back.
if os.environ.get("ATTN_MM_DT", "fp16") == "fp16":
    FULL.mm_dt = mybir.dt.float16
    FULL.np_dt = np.float16


def build_kernel(tc, ctx, cfg, xT, wq, wk, wv, wo, out, dbg=None):
    nc = tc.nc
    P = 128
    KC, NQB, QB, NKC, PAIRS = cfg.kc, cfg.nqb, cfg.qb, cfg.nkc, cfg.pairs
    MD = cfg.mm_dt
    SCALE = cfg.dim ** -0.5
    M_SLABS = cfg.shard // 128  # = PAIRS

    def r(ap):
        return ap

    wpool = ctx.enter_context(tc.tile_pool(name="w", bufs=1))
    wq_sb = wpool.tile([P, KC, cfg.shard], MD, tag="wq", name="wq_sb")
    wk_sb = wpool.tile([P, KC, cfg.shard], MD, tag="wk", name="wk_sb")
    wv_sb = wpool.tile([P, KC, cfg.vw], MD, tag="wv", name="wv_sb")
    wo_sb = wpool.tile([P, M_SLABS, cfg.dim], MD, tag="wo", name="wo_sb")

    per = ctx.enter_context(tc.tile_pool(name="per", bufs=1))
    qt = {}  # (pair, nqb) -> [128, QB]
    kt = {}
    vt = {}  # nt -> [128, vw]
    on = {}  # (slab, nqb) -> [128, QB]  normalized O^T for out-proj lhsT
    for g in range(PAIRS):
        for b in range(NQB):
            qt[g, b] = per.tile([P, QB], MD, tag=f"qt{g}_{b}", name=f"qt{g}_{b}")
            kt[g, b] = per.tile([P, QB], MD, tag=f"kt{g}_{b}", name=f"kt{g}_{b}")
            on[g, b] = per.tile([P, QB], MD, tag=f"on{g}_{b}", name=f"on{g}_{b}")
    for t in range(NKC):
        vt[t] = per.tile([P, cfg.vw], MD, tag=f"v{t}", name=f"v{t}")

    xpool = ctx.enter_context(tc.tile_pool(name="x", bufs=1))
    paQ = ctx.enter_context(tc.tile_pool(name="paQ", bufs=2, space="PSUM"))
    psS = ctx.enter_context(tc.tile_pool(name="psS", bufs=2, space="PSUM"))
    psO = ctx.enter_context(tc.tile_pool(name="psO", bufs=1, space="PSUM"))
    epool = ctx.enter_context(tc.tile_pool(name="e", bufs=10))
    npool = ctx.enter_context(tc.tile_pool(name="nrm", bufs=3))
    copool = ctx.enter_context(tc.tile_pool(name="co", bufs=6))

    xts = {}
    for b in range(NQB):
        xts[b] = xpool.tile([P, KC, QB], MD, tag=f"xt{b}", name=f"xt{b}")
    h = KC // 2
    nc.sync.dma_start(wk_sb[:, :h], wk[:, :h])
    nc.sync.dma_start(xts[0][:, :h], xT[:, :h, ts(0, QB)])
    nc.sync.dma_start(wk_sb[:, h:], wk[:, h:])
    nc.sync.dma_start(xts[0][:, h:], xT[:, h:, ts(0, QB)])
    nc.sync.dma_start(wq_sb[:], wq[:])
    for b in range(1, NQB):
        nc.sync.dma_start(xts[b][:], xT[:, :, ts(b, QB)])
    nc.sync.dma_start(wv_sb[:], wv[:])

    def emit_qk(w_sb, dst, g, b):
        ps = paQ.tile([P, 512], F32, tag="pa", name="pa")
        for kc in range(KC):
            nc.tensor.matmul(
                ps[:, :QB],
                lhsT=w_sb[:, kc, ts(g, 128)],
                rhs=xts[b][:, kc, :],
                start=(kc == 0),
                stop=(kc == KC - 1),
            )
        nc.vector.tensor_copy(dst[g, b][:], ps[:, :QB])

    def emit_v(nt):
        vb, t = divmod(nt, QB // 128)
        ps = paQ.tile([P, 512], F32, tag="pa", name="pa")
        for kc in range(KC):
            nc.tensor.matmul(
                ps[:, : cfg.vw],
                lhsT=xts[vb][:, kc, ts(t, 128)],
                rhs=wv_sb[:, kc, :],
                start=(kc == 0),
                stop=(kc == KC - 1),
            )
        nc.vector.tensor_copy(vt[nt][:], ps[:, : cfg.vw])
        v4 = vt[nt][:].rearrange("p (h e) -> p h e", e=DH + 1)
        ones_ap = v4[:, :, DH : DH + 1]
        if mybir.dt.size(MD) == 4:
            ones_ap = ones_ap.bitcast(F32)
        nc.vector.memset(ones_ap, 1.0)

    emitted = set()

    def attention(b, g, with_v=False, fillers=None, nfill=0, last=False):
        o_ps = psO.tile([P, 2, 512], F32, tag="o", name="o_ps")
        done = 0
        for c in range(NKC):
            if with_v:
                emit_v(c)
            if fillers and done < nfill and c in (5, 9, 13):
                key, fn = fillers.popleft()
                fn()
                emitted.add(key)
                done += 1
            s_ps = psS.tile([P, 2, 512], F32, tag="s", name="s_ps")
            for a in range(2):
                lo = a * 64
                nc.tensor.matmul(
                    s_ps[:, a, :QB],
                    lhsT=kt[g, c * 128 // QB][lo : lo + 64, ts(c % (QB // 128), 128)],
                    rhs=qt[g, b][lo : lo + 64, :],
                    start=True,
                    stop=True,
                )
            e_t = epool.tile([P, 2, 512], MD, tag="e", name="e_t")
            nc.scalar.activation(
                e_t[:, :, :QB],
                s_ps[:, :, :QB],
                mybir.ActivationFunctionType.Exp,
                scale=SCALE,
            )
            if dbg is not None and b == 0 and g == 0 and c == 0:
                nc.sync.dma_start(dbg["e00"][:], e_t[:, :, :QB])
            v4 = vt[c][:].rearrange("p (h e) -> p h e", e=DH + 1)
            for a in range(2):
                h = 2 * g + a
                nc.tensor.matmul(
                    o_ps[0 : DH + 1, a, :QB],
                    lhsT=v4[:, h, :],
                    rhs=e_t[:, a, :QB],
                    start=(c == 0),
                    stop=(c == NKC - 1),
                )
        # one-shot evacuation frees the O psum banks immediately; skipped on
        # the final sweep where nothing waits on the banks and the copy only
        # lengthens the tail-critical normalize chain
        if last:
            oev = o_ps
        else:
            oev = npool.tile([P, 2, 512], F32, tag="oev", name="oev")
            nc.vector.tensor_copy(oev[0 : DH + 1, :, :QB], o_ps[0 : DH + 1, :, :QB])
        if dbg is not None and b == 0 and g == 0:
            nc.sync.dma_start(dbg["oraw00"][:], oev[0 : DH + 1, :, :QB])
        # normalize; stage the denom row at partition 0 (the custom DVE
        # reciprocal misreads inputs at a nonzero base partition)
        drow = npool.tile([1, 2, 512], F32, tag="drow", name="drow")
        nc.vector.tensor_copy(drow[:, :, :QB], o_ps[DH : DH + 1, :, :QB])
        recip = npool.tile([1, 2, 512], F32, tag="recip", name="recip")
        nc.vector.reciprocal_approx_fast(out=recip[:, :, :QB], in_=drow[:, :, :QB])
        bcast = npool.tile([64, 2, 512], F32, tag="bcast", name="bcast")
        nc.gpsimd.partition_broadcast(bcast[:, :, :QB], recip[:, :, :QB])
        if dbg is not None and b == 0 and g == 0:
            nc.sync.dma_start(dbg["recip00"][:], recip[:, :, :QB])
            nc.sync.dma_start(dbg["bcast00"][:], bcast[:, :, :QB])
        for a in range(2):
            nc.vector.tensor_tensor(
                on[g, b][a * 64 : a * 64 + 64, :],
                oev[0:DH, a, :QB],
                bcast[:, a, :QB],
                mybir.AluOpType.mult,
            )

    # ---- emission schedule: attention starts early; remaining projections
    # are interleaved INSIDE sweeps so they fill PE slack without stalling ACT.
    # Emission order is program order: every filler (tile writer) must be
    # emitted before the first sweep that reads its tile.
    from collections import deque
    from functools import partial

    for b in range(NQB):
        emit_qk(wk_sb, kt, 0, b)
    emit_qk(wq_sb, qt, 0, 0)
    nc.sync.dma_start(wo_sb[:], wo[:])

    pend = deque()
    for b in range(1, NQB):
        pend.append((("q", 0, b), partial(emit_qk, wq_sb, qt, 0, b)))
    if PAIRS > 1:
        for b in range(NQB):
            pend.append((("k", 1, b), partial(emit_qk, wk_sb, kt, 1, b)))
        for b in range(NQB):
            pend.append((("q", 1, b), partial(emit_qk, wq_sb, qt, 1, b)))

    def fill_one():
        key, fn = pend.popleft()
        fn()
        emitted.add(key)

    def require(keys):
        while pend and any(k not in emitted for k in keys):
            fill_one()

    def sweep(b, g, **kw):
        keys = [("q", g, b)] if (g, b) != (0, 0) else []
        keys += [("k", g, bb) for bb in range(NQB)] if g > 0 else []
        require(keys)
        attention(b, g, fillers=pend, nfill=kw.pop("nfill", 0), **kw)

    attention(0, 0, with_v=True, fillers=pend, nfill=NQB - 1)
    for b in range(1, NQB):
        sweep(b, 0, nfill=2)
    def out_proj(bb):
        NH = cfg.dim // 512
        for t in range(QB // 128):
            nt = bb * (QB // 128) + t
            for nh in range(NH):
                ps = paQ.tile([P, 512], F32, tag="pa", name="pc")
                for kc in range(M_SLABS):
                    nc.tensor.matmul(
                        ps[:],
                        lhsT=on[kc, bb][:, ts(t, 128)],
                        rhs=wo_sb[:, kc, ts(nh, 512)],
                        start=(kc == 0),
                        stop=(kc == M_SLABS - 1),
                    )
                ot = copool.tile([P, 512], MD, tag="ot", name="ot")
                nc.vector.tensor_copy(ot[:], ps[:])
                nc.sync.dma_start(out[ts(nt, 128), ts(nh, 512)], ot[:])

    if PAIRS > 1:
        for b in range(NQB):
            sweep(b, 1, nfill=1, last=(b == NQB - 1))
    while pend:
        fill_one()
    for b in range(NQB):
        out_proj(b)

    if dbg is not None:
        nc.sync.dma_start(dbg["qt00"][:], qt[0, 0][:])
        nc.sync.dma_start(dbg["kt00"][:], kt[0, 0][:])
        nc.sync.dma_start(dbg["vt0"][:], vt[0][:])
        nc.sync.dma_start(dbg["on00"][:], on[0, 0][:])


def build_program(cfg, num_devices=N_CORES):
    nc = bacc.Bacc("TRN2", target_bir_lowering=False, debug=False, num_devices=num_devices)
    P = 128
    xT = nc.dram_tensor("xT", [P, cfg.kc, cfg.n], cfg.mm_dt, kind="ExternalInput").ap()
    wq = nc.dram_tensor("wq", [P, cfg.kc, cfg.shard], cfg.mm_dt, kind="ExternalInput").ap()
    wk = nc.dram_tensor("wk", [P, cfg.kc, cfg.shard], cfg.mm_dt, kind="ExternalInput").ap()
    wv = nc.dram_tensor("wv", [P, cfg.kc, cfg.vw], cfg.mm_dt, kind="ExternalInput").ap()
    wo = nc.dram_tensor("wo", [P, cfg.shard // 128, cfg.dim], cfg.mm_dt, kind="ExternalInput").ap()
    out = nc.dram_tensor("out", [cfg.n, cfg.dim], cfg.mm_dt, kind="ExternalOutput").ap()
    dbg = None
    if getattr(cfg, "dbg", False):
        QB = cfg.qb
        dbg = {
            "qt00": nc.dram_tensor("qt00", [P, QB], cfg.mm_dt, kind="ExternalOutput").ap(),
            "kt00": nc.dram_tensor("kt00", [P, QB], cfg.mm_dt, kind="ExternalOutput").ap(),
            "vt0": nc.dram_tensor("vt0", [P, cfg.vw], cfg.mm_dt, kind="ExternalOutput").ap(),
            "on00": nc.dram_tensor("on00", [P, QB], cfg.mm_dt, kind="ExternalOutput").ap(),
            "e00": nc.dram_tensor("e00", [P, 2, QB], cfg.mm_dt, kind="ExternalOutput").ap(),
            "oraw00": nc.dram_tensor("oraw00", [DH + 1, 2, QB], F32, kind="ExternalOutput").ap(),
            "recip00": nc.dram_tensor("recip00", [1, 2, QB], F32, kind="ExternalOutput").ap(),
            "bcast00": nc.dram_tensor("bcast00", [64, 2, QB], F32, kind="ExternalOutput").ap(),
        }
    with tile.TileContext(nc) as tc, ExitStack() as ctx:
        build_kernel(tc, ctx, cfg, xT, wq, wk, wv, wo, out, dbg=dbg)
    nc.compile()
    return nc


def shard_inputs(cfg, x, W_qkv, W_out, n_groups):
    """Build per-core input maps. Core c = (batch b, head-group g): c = b*n_groups + g."""
    b_sz = x.shape[0]
    dim, hg, sh = cfg.dim, cfg.hg, cfg.shard
    xTs = []
    for b in range(b_sz):
        xt = np.ascontiguousarray(
            x[b].T.reshape(cfg.kc, 128, cfg.n).transpose(1, 0, 2)
        )
        xTs.append(xt)

    def wlayout(w):  # [dim, C] -> [128, kc, C]
        return np.ascontiguousarray(
            w.reshape(cfg.kc, 128, w.shape[1]).transpose(1, 0, 2)
        )

    in_maps = []
    for b in range(b_sz):
        for g in range(n_groups):
            wq = W_qkv[:, sh * g : sh * (g + 1)]
            wk = W_qkv[:, dim + sh * g : dim + sh * (g + 1)]
            wv_cols = W_qkv[:, 2 * dim + sh * g : 2 * dim + sh * (g + 1)]
            wv = np.zeros((dim, cfg.vw), np.float32)
            for h in range(hg):
                wv[:, h * (DH + 1) : h * (DH + 1) + DH] = wv_cols[:, h * DH : (h + 1) * DH]
            wo = W_out[sh * g : sh * (g + 1), :]
            wo_l = np.ascontiguousarray(
                wo.reshape(sh // 128, 128, dim).transpose(1, 0, 2)
            )
            in_maps.append(
                {
                    "xT": xTs[b].astype(cfg.np_dt),
                    "wq": wlayout(wq).astype(cfg.np_dt),
                    "wk": wlayout(wk).astype(cfg.np_dt),
                    "wv": wlayout(wv).astype(cfg.np_dt),
                    "wo": wo_l.astype(cfg.np_dt),
                }
            )
    return in_maps


_NC_CACHE = {}


def kernel(x, W_qkv, W_out, b_out):
    x = np.asarray(x, np.float32)
    W_qkv = np.asarray(W_qkv, np.float32)
    W_out = np.asarray(W_out, np.float32)
    b_out = np.asarray(b_out, np.float32)
    cfg = FULL
    bsz = x.shape[0]
    n_groups = N_CORES // bsz

    if "nc" not in _NC_CACHE:
        _NC_CACHE["nc"] = build_program(cfg)
    nc = _NC_CACHE["nc"]

    in_maps = shard_inputs(cfg, x, W_qkv, W_out, n_groups)
    res = run_bass_kernel_spmd(nc, in_maps, list(range(N_CORES)))

    out = np.zeros((bsz, cfg.n, cfg.dim), np.float32)
    for b in range(bsz):
        for g in range(n_groups):
            out[b] += res.results[b * n_groups + g]["out"].astype(np.float32)
        out[b] += b_out
    return out
